# revision 6
# baseline (speedup 1.0000x reference)
"""Contrastive loss kernel for 8 TRN2 NeuronCores (Bass/Tile).

Algorithm (host sorts rows by class so same-class pairs are contiguous):
  loss*n = pos + neg
  pos = sum_c cnt_c^2 - sum_c ||v_c||^2          (class-sum embeddings; tiny matmul)
  neg = sum_ij sim*[sim>m_i] - (same-class correction over sorted windows)
      = sum_i relu(sim-m_i)            -> per-row accumulators (ACT relu+accum
                                          or DVE STT sub/max+accum), racc
      + sum_ij m_i*[sim>m_i]           -> G=is_gt(sA) tiles, m-weighted column
                                          sums via PE matmuls into one PSUM
                                          accumulator (mg)
      - corrections                    -> window matmul + 2 DVE STT+accum (wacc)

Per core: 8 row-chunks x 8 col-units of [128,1024] psum tiles. Scans split
ACT/DVE statically (A_COUNT). Counts ride the tensor engine (deferred
same-weight matmuls over buffered G tiles). Host does the O(n) reduction in
float64.
"""

import numpy as np
import ml_dtypes
from contextlib import ExitStack

import concourse.bacc as bacc
import concourse.mybir as mybir
import concourse.tile as tile
from concourse.bass_utils import run_bass_kernel_spmd

N, D, C = 8192, 128, 100
M = 8             # cores
RPC = N // M      # 1024 rows per core
NCH = RPC // 128  # 8 row-chunks per core
CW = 1024         # unit width (2 psum banks)
NU = N // CW      # 8 col-units per row-chunk
W = 512           # correction window width

A_COUNT = 44      # units scanned by ACT (of 64); rest by DVE


def _r_units():
    """Units whose count-sums go via DVE tensor_reduce instead of PE matmuls."""
    r = {u for u in range(NCH * NU) if u % NU in (2, 5)}
    r |= {u for u in range(NCH * NU) if u % NU == 7 and u // NU < 4}
    return r

BF16 = ml_dtypes.bfloat16

_nc_cache = None
LAST_RESULTS = None


def _a_units():
    """Spread A_COUNT ACT-scanned units evenly over the 64 units."""
    return {u for u in range(NCH * NU)
            if (u + 1) * A_COUNT // (NCH * NU) > u * A_COUNT // (NCH * NU)}


def _build_nc():
    f32 = mybir.dt.float32
    bf = mybir.dt.bfloat16
    A = mybir.ActivationFunctionType
    OP = mybir.AluOpType

    a_units = _a_units()
    r_units = _r_units()

    nc = bacc.Bacc("TRN2", target_bir_lowering=False, debug=False)

    xt = nc.dram_tensor("xt", [128, N], bf, kind="ExternalInput")        # X_sorted^T (full)
    xtl = nc.dram_tensor("xtl", [128, RPC], bf, kind="ExternalInput")    # core's rows, transposed
    xtw = nc.dram_tensor("xtw", [128, NCH * W], bf, kind="ExternalInput")  # correction windows
    xsr = nc.dram_tensor("xsr", [RPC, 128], bf, kind="ExternalInput")    # core's rows, untransposed
    mrow = nc.dram_tensor("mrow", [128, NCH], f32, kind="ExternalInput")
    trow = nc.dram_tensor("trow", [128, NCH], f32, kind="ExternalInput")
    eqm = nc.dram_tensor("eqm", [128, NCH * W], bf, kind="ExternalInput")
    iotab = nc.dram_tensor("iotab", [128, C], f32, kind="ExternalInput")
    out_racc = nc.dram_tensor("out_racc", [128, NCH * NU], f32, kind="ExternalOutput")
    out_cacc = nc.dram_tensor("out_cacc", [128, NCH * NU], f32, kind="ExternalOutput")
    out_wacc = nc.dram_tensor("out_wacc", [128, 2 * NCH], f32, kind="ExternalOutput")
    out_mg = nc.dram_tensor("out_mg", [1, 512], f32, kind="ExternalOutput")
    out_v = nc.dram_tensor("out_v", [C, 128], f32, kind="ExternalOutput")

    with tile.TileContext(nc) as tc, ExitStack() as ctx:
        consts = ctx.enter_context(tc.tile_pool(name="consts", bufs=1))
        scratch = ctx.enter_context(tc.tile_pool(name="scratch", bufs=4))
        gpool = ctx.enter_context(tc.tile_pool(name="gpool", bufs=2))

        dma = nc.default_dma_engine

        xtl_sb = consts.tile([128, RPC], bf)
        dma.dma_start(out=xtl_sb[:], in_=xtl[:])
        xt_sb = consts.tile([128, N], bf)
        for p in range(4):
            s = p * (N // 4)
            dma.dma_start(out=xt_sb[:, s:s + N // 4], in_=xt[:, s:s + N // 4])
        m_sb = consts.tile([128, NCH], f32)
        dma.dma_start(out=m_sb[:], in_=mrow[:])
        t_sb = consts.tile([128, NCH], f32)
        dma.dma_start(out=t_sb[:], in_=trow[:])
        xtw_sb = consts.tile([128, NCH * W], bf)
        dma.dma_start(out=xtw_sb[:], in_=xtw[:])
        eqm_sb = consts.tile([128, NCH * W], bf)
        dma.dma_start(out=eqm_sb[:], in_=eqm[:])
        io_sb = consts.tile([128, C], f32)
        dma.dma_start(out=io_sb[:], in_=iotab[:])
        xs_sb = consts.tile([128, NCH, 128], bf)
        for ch in range(NCH):
            dma.dma_start(out=xs_sb[:, ch, :], in_=xsr[ch * 128:(ch + 1) * 128, :])

        negm = consts.tile([128, NCH], f32)
        nc.vector.tensor_scalar_mul(negm[:], m_sb[:], -1.0)
        mbf = consts.tile([128, NCH], bf)     # bf16 margins: weights for m*G sums
        nc.vector.tensor_copy(mbf[:], m_sb[:])
        zeros = consts.tile([128, CW], bf)
        nc.vector.memset(zeros[:], 0.0)

        st_all = consts.tile([128, NCH, C], bf)   # one-hot (class == target) per row-chunk
        racc = consts.tile([128, NCH * NU], f32)
        cacc = consts.tile([128, NCH * NU], f32)
        nc.vector.memset(cacc[:], 0.0)
        wacc = consts.tile([128, 2 * NCH], f32)

        with tc.tile_pool(name="psA", bufs=3, space="PSUM") as psum, \
             tc.tile_pool(name="psacc", bufs=1, space="PSUM") as psacc:
            accm = psacc.tile([1, 512], mybir.dt.float32, tag="accm")

            gbufs = [None] * NCH
            pe_blocks = [(ch, q) for ch in range(NCH) for q in range(N // 512)
                         if (ch * NU + q // 2) not in r_units]
            first_blk, last_blk = pe_blocks[0], pe_blocks[-1]

            def emit_cnt(ch, q):
                # m-weighted count sums for chunk ch, 512-col block q
                if (ch * NU + q // 2) in r_units:
                    return
                nc.tensor.matmul(accm[:], mbf[:, ch:ch + 1],
                                 gbufs[ch][:, q * 512:(q + 1) * 512],
                                 start=((ch, q) == first_blk),
                                 stop=((ch, q) == last_blk),
                                 skip_group_check=True)

            for ch in range(NCH):
                lhsT = xtl_sb[:, ch * 128:(ch + 1) * 128]
                mcol = m_sb[:, ch:ch + 1]
                ncol = negm[:, ch:ch + 1]
                gbuf = gpool.tile([128, N], bf, tag="gbuf")
                gbufs[ch] = gbuf

                for j2 in range(NU):
                    unit = ch * NU + j2
                    ps = psum.tile([128, CW], mybir.dt.float32, tag="ps")
                    for q in range(CW // 512):
                        j = j2 * (CW // 512) + q
                        nc.tensor.matmul(ps[:, q * 512:(q + 1) * 512], lhsT,
                                         xt_sb[:, j * 512:(j + 1) * 512],
                                         start=True, stop=True)
                    if ch > 0:
                        # pipeline: previous chunk's count matmuls interleave
                        emit_cnt(ch - 1, 2 * j2)
                        emit_cnt(ch - 1, 2 * j2 + 1)
                    sA = scratch.tile([128, CW], bf, tag="sA")
                    if unit in a_units:
                        nc.scalar.activation(sA[:], ps[:], A.Relu,
                                             bias=ncol, scale=1.0,
                                             accum_out=racc[:, unit:unit + 1])
                    else:
                        nc.vector.scalar_tensor_tensor(
                            sA[:], ps[:], mcol, zeros[:],
                            OP.subtract, OP.max,
                            accum_out=racc[:, unit:unit + 1])
                    nc.vector.tensor_scalar(gbufs[ch][:, j2 * CW:(j2 + 1) * CW],
                                            sA[:], 0.0, None, OP.is_gt)
                    if unit in r_units:
                        nc.vector.tensor_reduce(
                            cacc[:, unit:unit + 1],
                            gbufs[ch][:, j2 * CW:(j2 + 1) * CW],
                            axis=mybir.AxisListType.X, op=OP.add)

                # same-class correction over this row-chunk's sorted window
                wps = psum.tile([128, CW], mybir.dt.float32, tag="ps")
                nc.tensor.matmul(wps[:, 0:W], lhsT,
                                 xtw_sb[:, ch * W:(ch + 1) * W],
                                 start=True, stop=True)
                uw = scratch.tile([128, W], bf, tag="uw")
                nc.scalar.activation(uw[:], wps[:, 0:W], A.Relu,
                                     bias=ncol, scale=1.0)
                eqs = eqm_sb[:, ch * W:(ch + 1) * W]
                jk1 = scratch.tile([128, W], bf, tag="jk1")
                nc.vector.scalar_tensor_tensor(
                    jk1[:], uw[:], 1.0, eqs, OP.mult, OP.mult,
                    accum_out=wacc[:, 2 * ch:2 * ch + 1])
                jk2 = scratch.tile([128, W], bf, tag="jk2")
                nc.vector.scalar_tensor_tensor(
                    jk2[:], uw[:], 0.0, eqs, OP.is_gt, OP.mult,
                    accum_out=wacc[:, 2 * ch + 1:2 * ch + 2])

                nc.vector.tensor_scalar(st_all[:, ch, :], io_sb[:],
                                        t_sb[:, ch:ch + 1], None, OP.is_equal)

            for q in range(N // 512):
                emit_cnt(NCH - 1, q)

            mg_sb = consts.tile([1, 512], f32)
            nc.scalar.copy(mg_sb[:], accm[:])

        # partial class-sum vectors V_k = S^T X over this core's rows
        v_sb = consts.tile([C, 128], f32)
        with tc.tile_pool(name="psB", bufs=1, space="PSUM") as vpsum:
            v_ps = vpsum.tile([C, 128], mybir.dt.float32)
            for ch in range(NCH):
                nc.tensor.matmul(v_ps[:], st_all[:, ch, :], xs_sb[:, ch, :],
                                 start=(ch == 0), stop=(ch == NCH - 1))
            nc.scalar.copy(v_sb[:], v_ps[:])

        dma.dma_start(out=out_v[:], in_=v_sb[:])
        dma.dma_start(out=out_racc[:], in_=racc[:])
        dma.dma_start(out=out_cacc[:], in_=cacc[:])
        dma.dma_start(out=out_wacc[:], in_=wacc[:])
        dma.dma_start(out=out_mg[:], in_=mg_sb[:])

    nc.compile()
    return nc


def _prep(inputs, margin, targets):
    """Host-side sharding/layout prep. Returns per-core input maps + reduction data."""
    t = np.asarray(targets).astype(np.int64)
    x = np.asarray(inputs, dtype=np.float32)
    m = np.asarray(margin, dtype=np.float32)

    perm = np.argsort(t, kind="stable")
    xs, ms, ts = x[perm], m[perm], t[perm]
    x_bf = xs.astype(BF16)
    xt_bf = np.ascontiguousarray(x_bf.T)          # [128, N]

    cnt = np.bincount(ts, minlength=C).astype(np.float64)
    starts = np.concatenate([[0], np.cumsum(np.bincount(ts, minlength=C))]).astype(np.int64)

    nchunks = N // 128
    wstart = np.zeros(nchunks, np.int64)
    for g in range(nchunks):
        lo, hi = ts[g * 128], ts[g * 128 + 127]
        width = starts[hi + 1] - starts[lo]
        assert width <= W - 2, f"class window {width} too wide for chunk {g}"
        w0 = min(int(starts[lo]), N - W) & ~1
        wstart[g] = w0

    iotab = np.ascontiguousarray(
        np.broadcast_to(np.arange(C, dtype=np.float32), (128, C)))

    in_maps = []
    mrows = []
    for k in range(M):
        r0 = k * RPC
        g0 = r0 // 128
        mrow = np.ascontiguousarray(ms[r0:r0 + RPC].reshape(NCH, 128).T)
        trowf = np.ascontiguousarray(ts[r0:r0 + RPC].reshape(NCH, 128).T.astype(np.float32))
        xtw = np.concatenate(
            [xt_bf[:, wstart[g0 + ch]:wstart[g0 + ch] + W] for ch in range(NCH)], axis=1)
        eqm = np.concatenate(
            [(ts[r0 + ch * 128:r0 + (ch + 1) * 128, None]
              == ts[None, wstart[g0 + ch]:wstart[g0 + ch] + W]).astype(BF16)
             for ch in range(NCH)], axis=1)
        in_maps.append({
            "xt": xt_bf,
            "xtl": np.ascontiguousarray(xt_bf[:, r0:r0 + RPC]),
            "xtw": np.ascontiguousarray(xtw),
            "xsr": np.ascontiguousarray(x_bf[r0:r0 + RPC]),
            "mrow": mrow,
            "trow": trowf,
            "eqm": np.ascontiguousarray(eqm),
            "iotab": iotab,
        })
        mrows.append(mrow.astype(np.float64))
    return in_maps, mrows, cnt


def kernel(inputs, margin, targets):
    global _nc_cache, LAST_RESULTS
    in_maps, mrows, cnt = _prep(inputs, margin, targets)
    if _nc_cache is None:
        _nc_cache = _build_nc()
    res = run_bass_kernel_spmd(_nc_cache, in_maps, list(range(M)))
    LAST_RESULTS = res

    neg = 0.0
    V = np.zeros((C, 128), np.float64)
    for k in range(M):
        r = res.results[k]
        mr = mrows[k]                                  # [128, NCH] fp64
        neg += r["out_racc"].astype(np.float64).sum()  # sum relu(sim - m)
        neg += r["out_mg"].astype(np.float64).sum()    # sum m_i * [sim > m_i] (PE units)
        ca = r["out_cacc"].astype(np.float64)          # per-row counts (R units)
        neg += (np.repeat(mr, NU, axis=1) * ca).sum()
        wa = r["out_wacc"].astype(np.float64)          # [128, 2*NCH]
        neg -= wa[:, 0::2].sum()                       # same-class relu correction
        neg -= (mr * wa[:, 1::2]).sum()                # same-class m*count correction
        V += r["out_v"].astype(np.float64)

    pos = (cnt ** 2).sum() - (V ** 2).sum()
    loss = (pos + neg) / N
    return np.float32(loss)


# revision 7
# speedup vs baseline: 1.1140x; 1.1140x over previous
"""Contrastive loss kernel for 8 TRN2 NeuronCores (Bass/Tile).

Algorithm (host sorts rows by class so same-class pairs are contiguous):
  loss*n = pos + neg
  pos = sum_c cnt_c^2 - sum_c ||v_c||^2          (class-sum embeddings; tiny matmul)
  neg = sum_ij sim*[sim>m_i] - (same-class correction over sorted windows)
      = sum_i relu(sim-m_i)            -> per-row accumulators (ACT relu+accum
                                          or DVE STT sub/max+accum), racc
      + sum_ij m_i*[sim>m_i]           -> G=is_gt(sA) tiles, m-weighted column
                                          sums via PE matmuls into one PSUM
                                          accumulator (mg)
      - corrections                    -> window matmul + 2 DVE STT+accum (wacc)

Per core: 8 row-chunks x 8 col-units of [128,1024] psum tiles. Scans split
ACT/DVE statically (A_COUNT). Counts ride the tensor engine (deferred
same-weight matmuls over buffered G tiles). Host does the O(n) reduction in
float64.
"""

import numpy as np
import ml_dtypes
from contextlib import ExitStack

import concourse.bacc as bacc
import concourse.mybir as mybir
import concourse.tile as tile
from concourse.bass_utils import run_bass_kernel_spmd

N, D, C = 8192, 128, 100
M = 8             # cores
RPC = N // M      # 1024 rows per core
NCH = RPC // 128  # 8 row-chunks per core
CW = 1024         # unit width (2 psum banks)
NU = N // CW      # 8 col-units per row-chunk
W = 512           # correction window width

A_COUNT = 41      # units scanned by ACT (of 64); rest by DVE


def _r_units():
    """Units whose count-sums go via DVE tensor_reduce instead of PE matmuls."""
    return set()

BF16 = ml_dtypes.bfloat16

_nc_cache = None
LAST_RESULTS = None


def _a_units():
    """Spread A_COUNT ACT-scanned units evenly over the 64 units."""
    return {u for u in range(NCH * NU)
            if (u + 1) * A_COUNT // (NCH * NU) > u * A_COUNT // (NCH * NU)}


def _build_nc():
    f32 = mybir.dt.float32
    bf = mybir.dt.bfloat16
    A = mybir.ActivationFunctionType
    OP = mybir.AluOpType

    a_units = _a_units()
    r_units = _r_units()

    nc = bacc.Bacc("TRN2", target_bir_lowering=False, debug=False)

    xt = nc.dram_tensor("xt", [128, N], bf, kind="ExternalInput")        # X_sorted^T (full)
    xtl = nc.dram_tensor("xtl", [128, RPC], bf, kind="ExternalInput")    # core's rows, transposed
    xtw = nc.dram_tensor("xtw", [128, NCH * W], bf, kind="ExternalInput")  # correction windows
    xsr = nc.dram_tensor("xsr", [RPC, 128], bf, kind="ExternalInput")    # core's rows, untransposed
    mrow = nc.dram_tensor("mrow", [128, NCH], f32, kind="ExternalInput")
    trow = nc.dram_tensor("trow", [128, NCH], f32, kind="ExternalInput")
    eqm = nc.dram_tensor("eqm", [128, NCH * W], bf, kind="ExternalInput")
    iotab = nc.dram_tensor("iotab", [128, C], f32, kind="ExternalInput")
    out_racc = nc.dram_tensor("out_racc", [128, NCH * NU], f32, kind="ExternalOutput")
    out_cacc = nc.dram_tensor("out_cacc", [128, NCH * NU], f32, kind="ExternalOutput")
    out_wacc = nc.dram_tensor("out_wacc", [128, 2 * NCH], f32, kind="ExternalOutput")
    out_mg = nc.dram_tensor("out_mg", [1, 512], f32, kind="ExternalOutput")
    out_v = nc.dram_tensor("out_v", [C, 128], f32, kind="ExternalOutput")

    with tile.TileContext(nc) as tc, ExitStack() as ctx:
        consts = ctx.enter_context(tc.tile_pool(name="consts", bufs=1))
        scratch = ctx.enter_context(tc.tile_pool(name="scratch", bufs=4))
        gpool = ctx.enter_context(tc.tile_pool(name="gpool", bufs=2))

        dma = nc.default_dma_engine

        xtl_sb = consts.tile([128, RPC], bf)
        dma.dma_start(out=xtl_sb[:], in_=xtl[:])
        xt_sb = consts.tile([128, N], bf)
        for p in range(4):
            s = p * (N // 4)
            dma.dma_start(out=xt_sb[:, s:s + N // 4], in_=xt[:, s:s + N // 4])
        m_sb = consts.tile([128, NCH], f32)
        dma.dma_start(out=m_sb[:], in_=mrow[:])
        t_sb = consts.tile([128, NCH], f32)
        dma.dma_start(out=t_sb[:], in_=trow[:])
        xtw_sb = consts.tile([128, NCH * W], bf)
        dma.dma_start(out=xtw_sb[:], in_=xtw[:])
        eqm_sb = consts.tile([128, NCH * W], bf)
        dma.dma_start(out=eqm_sb[:], in_=eqm[:])
        io_sb = consts.tile([128, C], f32)
        dma.dma_start(out=io_sb[:], in_=iotab[:])
        xs_sb = consts.tile([128, NCH, 128], bf)
        for ch in range(NCH):
            dma.dma_start(out=xs_sb[:, ch, :], in_=xsr[ch * 128:(ch + 1) * 128, :])

        negm = consts.tile([128, NCH], f32)
        nc.vector.tensor_scalar_mul(negm[:], m_sb[:], -1.0)
        mbf = consts.tile([128, NCH], bf)     # bf16 margins: weights for m*G sums
        nc.vector.tensor_copy(mbf[:], m_sb[:])
        zeros = consts.tile([128, CW], bf)
        nc.vector.memset(zeros[:], 0.0)

        st_all = consts.tile([128, NCH, C], bf)   # one-hot (class == target) per row-chunk
        racc = consts.tile([128, NCH * NU], f32)
        cacc = consts.tile([128, NCH * NU], f32)
        nc.vector.memset(cacc[:], 0.0)
        wacc = consts.tile([128, 2 * NCH], f32)

        with tc.tile_pool(name="psA", bufs=3, space="PSUM") as psum, \
             tc.tile_pool(name="psacc", bufs=1, space="PSUM") as psacc:
            accm = psacc.tile([1, 512], mybir.dt.float32, tag="accm")

            gbufs = [None] * NCH
            pe_blocks = [(ch, q) for ch in range(NCH) for q in range(N // 512)
                         if (ch * NU + q // 2) not in r_units]
            first_blk, last_blk = pe_blocks[0], pe_blocks[-1]

            def emit_cnt(ch, q):
                # m-weighted count sums for chunk ch, 512-col block q
                if (ch * NU + q // 2) in r_units:
                    return
                nc.tensor.matmul(accm[:], mbf[:, ch:ch + 1],
                                 gbufs[ch][:, q * 512:(q + 1) * 512],
                                 start=((ch, q) == first_blk),
                                 stop=((ch, q) == last_blk),
                                 skip_group_check=True)

            for ch in range(NCH):
                lhsT = xtl_sb[:, ch * 128:(ch + 1) * 128]
                mcol = m_sb[:, ch:ch + 1]
                ncol = negm[:, ch:ch + 1]
                gbuf = gpool.tile([128, N], bf, tag="gbuf")
                gbufs[ch] = gbuf

                for j2 in range(NU):
                    unit = ch * NU + j2
                    ps = psum.tile([128, CW], mybir.dt.float32, tag="ps")
                    for q in range(CW // 512):
                        j = j2 * (CW // 512) + q
                        nc.tensor.matmul(ps[:, q * 512:(q + 1) * 512], lhsT,
                                         xt_sb[:, j * 512:(j + 1) * 512],
                                         start=True, stop=True)
                    if ch > 0:
                        # pipeline: previous chunk's count matmuls interleave
                        emit_cnt(ch - 1, 2 * j2)
                        emit_cnt(ch - 1, 2 * j2 + 1)
                    sA = scratch.tile([128, CW], bf, tag="sA")
                    if unit in a_units:
                        nc.scalar.activation(sA[:], ps[:], A.Relu,
                                             bias=ncol, scale=1.0,
                                             accum_out=racc[:, unit:unit + 1])
                    else:
                        nc.vector.scalar_tensor_tensor(
                            sA[:], ps[:], mcol, zeros[:],
                            OP.subtract, OP.max,
                            accum_out=racc[:, unit:unit + 1])
                    nc.vector.tensor_scalar(gbufs[ch][:, j2 * CW:(j2 + 1) * CW],
                                            sA[:], 0.0, None, OP.is_gt)
                    if unit in r_units:
                        nc.vector.tensor_reduce(
                            cacc[:, unit:unit + 1],
                            gbufs[ch][:, j2 * CW:(j2 + 1) * CW],
                            axis=mybir.AxisListType.X, op=OP.add)

                # same-class correction over this row-chunk's sorted window
                wps = psum.tile([128, CW], mybir.dt.float32, tag="ps")
                nc.tensor.matmul(wps[:, 0:W], lhsT,
                                 xtw_sb[:, ch * W:(ch + 1) * W],
                                 start=True, stop=True)
                uw = scratch.tile([128, W], bf, tag="uw")
                nc.scalar.activation(uw[:], wps[:, 0:W], A.Relu,
                                     bias=ncol, scale=1.0)
                eqs = eqm_sb[:, ch * W:(ch + 1) * W]
                jk1 = scratch.tile([128, W], bf, tag="jk1")
                nc.vector.scalar_tensor_tensor(
                    jk1[:], uw[:], 1.0, eqs, OP.mult, OP.mult,
                    accum_out=wacc[:, 2 * ch:2 * ch + 1])
                jk2 = scratch.tile([128, W], bf, tag="jk2")
                nc.vector.scalar_tensor_tensor(
                    jk2[:], uw[:], 0.0, eqs, OP.is_gt, OP.mult,
                    accum_out=wacc[:, 2 * ch + 1:2 * ch + 2])

                nc.vector.tensor_scalar(st_all[:, ch, :], io_sb[:],
                                        t_sb[:, ch:ch + 1], None, OP.is_equal)

            for q in range(N // 512):
                emit_cnt(NCH - 1, q)

            mg_sb = consts.tile([1, 512], f32)
            nc.scalar.copy(mg_sb[:], accm[:])

        # partial class-sum vectors V_k = S^T X over this core's rows
        v_sb = consts.tile([C, 128], f32)
        with tc.tile_pool(name="psB", bufs=1, space="PSUM") as vpsum:
            v_ps = vpsum.tile([C, 128], mybir.dt.float32)
            for ch in range(NCH):
                nc.tensor.matmul(v_ps[:], st_all[:, ch, :], xs_sb[:, ch, :],
                                 start=(ch == 0), stop=(ch == NCH - 1))
            nc.scalar.copy(v_sb[:], v_ps[:])

        dma.dma_start(out=out_v[:], in_=v_sb[:])
        dma.dma_start(out=out_racc[:], in_=racc[:])
        dma.dma_start(out=out_cacc[:], in_=cacc[:])
        dma.dma_start(out=out_wacc[:], in_=wacc[:])
        dma.dma_start(out=out_mg[:], in_=mg_sb[:])

    nc.compile()
    return nc


def _prep(inputs, margin, targets):
    """Host-side sharding/layout prep. Returns per-core input maps + reduction data."""
    t = np.asarray(targets).astype(np.int64)
    x = np.asarray(inputs, dtype=np.float32)
    m = np.asarray(margin, dtype=np.float32)

    perm = np.argsort(t, kind="stable")
    xs, ms, ts = x[perm], m[perm], t[perm]
    x_bf = xs.astype(BF16)
    xt_bf = np.ascontiguousarray(x_bf.T)          # [128, N]

    cnt = np.bincount(ts, minlength=C).astype(np.float64)
    starts = np.concatenate([[0], np.cumsum(np.bincount(ts, minlength=C))]).astype(np.int64)

    nchunks = N // 128
    wstart = np.zeros(nchunks, np.int64)
    for g in range(nchunks):
        lo, hi = ts[g * 128], ts[g * 128 + 127]
        width = starts[hi + 1] - starts[lo]
        assert width <= W - 2, f"class window {width} too wide for chunk {g}"
        w0 = min(int(starts[lo]), N - W) & ~1
        wstart[g] = w0

    iotab = np.ascontiguousarray(
        np.broadcast_to(np.arange(C, dtype=np.float32), (128, C)))

    in_maps = []
    mrows = []
    for k in range(M):
        r0 = k * RPC
        g0 = r0 // 128
        mrow = np.ascontiguousarray(ms[r0:r0 + RPC].reshape(NCH, 128).T)
        trowf = np.ascontiguousarray(ts[r0:r0 + RPC].reshape(NCH, 128).T.astype(np.float32))
        xtw = np.concatenate(
            [xt_bf[:, wstart[g0 + ch]:wstart[g0 + ch] + W] for ch in range(NCH)], axis=1)
        eqm = np.concatenate(
            [(ts[r0 + ch * 128:r0 + (ch + 1) * 128, None]
              == ts[None, wstart[g0 + ch]:wstart[g0 + ch] + W]).astype(BF16)
             for ch in range(NCH)], axis=1)
        in_maps.append({
            "xt": xt_bf,
            "xtl": np.ascontiguousarray(xt_bf[:, r0:r0 + RPC]),
            "xtw": np.ascontiguousarray(xtw),
            "xsr": np.ascontiguousarray(x_bf[r0:r0 + RPC]),
            "mrow": mrow,
            "trow": trowf,
            "eqm": np.ascontiguousarray(eqm),
            "iotab": iotab,
        })
        mrows.append(mrow.astype(np.float64))
    return in_maps, mrows, cnt


def kernel(inputs, margin, targets):
    global _nc_cache, LAST_RESULTS
    in_maps, mrows, cnt = _prep(inputs, margin, targets)
    if _nc_cache is None:
        _nc_cache = _build_nc()
    res = run_bass_kernel_spmd(_nc_cache, in_maps, list(range(M)))
    LAST_RESULTS = res

    neg = 0.0
    V = np.zeros((C, 128), np.float64)
    for k in range(M):
        r = res.results[k]
        mr = mrows[k]                                  # [128, NCH] fp64
        neg += r["out_racc"].astype(np.float64).sum()  # sum relu(sim - m)
        neg += r["out_mg"].astype(np.float64).sum()    # sum m_i * [sim > m_i] (PE units)
        ca = r["out_cacc"].astype(np.float64)          # per-row counts (R units)
        neg += (np.repeat(mr, NU, axis=1) * ca).sum()
        wa = r["out_wacc"].astype(np.float64)          # [128, 2*NCH]
        neg -= wa[:, 0::2].sum()                       # same-class relu correction
        neg -= (mr * wa[:, 1::2]).sum()                # same-class m*count correction
        V += r["out_v"].astype(np.float64)

    pos = (cnt ** 2).sum() - (V ** 2).sum()
    loss = (pos + neg) / N
    return np.float32(loss)


# revision 8
# speedup vs baseline: 1.1400x; 1.0233x over previous
"""Contrastive loss kernel for 8 TRN2 NeuronCores (Bass/Tile).

Algorithm (host sorts rows by class so same-class pairs are contiguous):
  loss*n = pos + neg
  pos = sum_c cnt_c^2 - sum_c ||v_c||^2          (class-sum embeddings; tiny matmul)
  neg = sum_ij sim*[sim>m_i] - (same-class correction over sorted windows)
      = sum_i relu(sim-m_i)            -> per-row accumulators (ACT relu+accum
                                          or DVE STT sub/max+accum), racc
      + sum_ij m_i*[sim>m_i]           -> G=is_gt(sA) tiles, m-weighted column
                                          sums via PE matmuls into one PSUM
                                          accumulator (mg)
      - corrections                    -> window matmul + 2 DVE STT+accum (wacc)

Per core: 8 row-chunks x 8 col-units of [128,1024] psum tiles. Scans split
ACT/DVE statically (A_COUNT). Counts ride the tensor engine (deferred
same-weight matmuls over buffered G tiles). Host does the O(n) reduction in
float64.
"""

import numpy as np
import ml_dtypes
from contextlib import ExitStack

import concourse.bacc as bacc
import concourse.mybir as mybir
import concourse.tile as tile
from concourse.bass_utils import run_bass_kernel_spmd

N, D, C = 8192, 128, 100
M = 8             # cores
RPC = N // M      # 1024 rows per core
NCH = RPC // 128  # 8 row-chunks per core
CW = 1024         # unit width (2 psum banks)
NU = N // CW      # 8 col-units per row-chunk
W = 512           # correction window width

A_COUNT = 41      # units scanned by ACT (of 64); rest by DVE


def _r_units():
    """Units whose count-sums go via DVE tensor_reduce instead of PE matmuls."""
    return set()

BF16 = ml_dtypes.bfloat16

_nc_cache = None
LAST_RESULTS = None


def _a_units():
    """Spread A_COUNT ACT-scanned units evenly over the 64 units."""
    return {u for u in range(NCH * NU)
            if (u + 1) * A_COUNT // (NCH * NU) > u * A_COUNT // (NCH * NU)}


def _build_nc():
    f32 = mybir.dt.float32
    bf = mybir.dt.bfloat16
    A = mybir.ActivationFunctionType
    OP = mybir.AluOpType

    a_units = _a_units()
    r_units = _r_units()

    nc = bacc.Bacc("TRN2", target_bir_lowering=False, debug=False)

    xt = nc.dram_tensor("xt", [128, N], bf, kind="ExternalInput")        # X_sorted^T (full)
    xtl = nc.dram_tensor("xtl", [128, RPC], bf, kind="ExternalInput")    # core's rows, transposed
    xtw = nc.dram_tensor("xtw", [128, NCH * W], bf, kind="ExternalInput")  # correction windows
    xsr = nc.dram_tensor("xsr", [RPC, 128], bf, kind="ExternalInput")    # core's rows, untransposed
    mrow = nc.dram_tensor("mrow", [128, NCH], f32, kind="ExternalInput")
    trow = nc.dram_tensor("trow", [128, NCH], f32, kind="ExternalInput")
    eqm = nc.dram_tensor("eqm", [128, NCH * W], bf, kind="ExternalInput")
    iotab = nc.dram_tensor("iotab", [128, C], f32, kind="ExternalInput")
    out_racc = nc.dram_tensor("out_racc", [128, NCH * NU], f32, kind="ExternalOutput")
    out_wacc = nc.dram_tensor("out_wacc", [128, 2 * NCH], f32, kind="ExternalOutput")
    out_mg = nc.dram_tensor("out_mg", [1, 512], f32, kind="ExternalOutput")
    out_v = nc.dram_tensor("out_v", [C, 128], f32, kind="ExternalOutput")

    with tile.TileContext(nc) as tc, ExitStack() as ctx:
        consts = ctx.enter_context(tc.tile_pool(name="consts", bufs=1))
        scratch = ctx.enter_context(tc.tile_pool(name="scratch", bufs=4))
        gpool = ctx.enter_context(tc.tile_pool(name="gpool", bufs=2))

        dma = nc.default_dma_engine

        xtl_sb = consts.tile([128, RPC], bf)
        dma.dma_start(out=xtl_sb[:], in_=xtl[:])
        xt_sb = consts.tile([128, N], bf)
        for p in range(4):
            s = p * (N // 4)
            dma.dma_start(out=xt_sb[:, s:s + N // 4], in_=xt[:, s:s + N // 4])
        m_sb = consts.tile([128, NCH], f32)
        dma.dma_start(out=m_sb[:], in_=mrow[:])
        t_sb = consts.tile([128, NCH], f32)
        dma.dma_start(out=t_sb[:], in_=trow[:])
        xtw_sb = consts.tile([128, NCH * W], bf)
        dma.dma_start(out=xtw_sb[:], in_=xtw[:])
        eqm_sb = consts.tile([128, NCH * W], bf)
        dma.dma_start(out=eqm_sb[:], in_=eqm[:])
        io_sb = consts.tile([128, C], f32)
        dma.dma_start(out=io_sb[:], in_=iotab[:])
        xs_sb = consts.tile([128, NCH, 128], bf)
        for ch in range(NCH):
            dma.dma_start(out=xs_sb[:, ch, :], in_=xsr[ch * 128:(ch + 1) * 128, :])

        negm = consts.tile([128, NCH], f32)
        nc.vector.tensor_scalar_mul(negm[:], m_sb[:], -1.0)
        mbf = consts.tile([128, NCH], bf)     # bf16 margins: weights for m*G sums
        nc.vector.tensor_copy(mbf[:], m_sb[:])
        zeros = consts.tile([128, CW], bf)
        nc.vector.memset(zeros[:], 0.0)

        st_all = consts.tile([128, NCH, C], bf)   # one-hot (class == target) per row-chunk
        racc = consts.tile([128, NCH * NU], f32)
        wacc = consts.tile([128, 2 * NCH], f32)

        with tc.tile_pool(name="psA", bufs=3, space="PSUM") as psum, \
             tc.tile_pool(name="psacc", bufs=1, space="PSUM") as psacc:
            accm = psacc.tile([1, 512], mybir.dt.float32, tag="accm")

            gbufs = [None] * NCH
            pe_blocks = [(ch, q) for ch in range(NCH) for q in range(N // 512)
                         if (ch * NU + q // 2) not in r_units]
            first_blk, last_blk = pe_blocks[0], pe_blocks[-1]

            def emit_cnt(ch, q):
                # m-weighted count sums for chunk ch, 512-col block q
                if (ch * NU + q // 2) in r_units:
                    return
                nc.tensor.matmul(accm[:], mbf[:, ch:ch + 1],
                                 gbufs[ch][:, q * 512:(q + 1) * 512],
                                 start=((ch, q) == first_blk),
                                 stop=((ch, q) == last_blk),
                                 skip_group_check=True)

            for ch in range(NCH):
                lhsT = xtl_sb[:, ch * 128:(ch + 1) * 128]
                mcol = m_sb[:, ch:ch + 1]
                ncol = negm[:, ch:ch + 1]
                gbuf = gpool.tile([128, N], bf, tag="gbuf")
                gbufs[ch] = gbuf

                for j2 in range(NU):
                    unit = ch * NU + j2
                    ps = psum.tile([128, CW], mybir.dt.float32, tag="ps")
                    for q in range(CW // 512):
                        j = j2 * (CW // 512) + q
                        nc.tensor.matmul(ps[:, q * 512:(q + 1) * 512], lhsT,
                                         xt_sb[:, j * 512:(j + 1) * 512],
                                         start=True, stop=True)
                    if ch > 0:
                        # pipeline: previous chunk's count matmuls interleave
                        emit_cnt(ch - 1, 2 * j2)
                        emit_cnt(ch - 1, 2 * j2 + 1)
                    sA = scratch.tile([128, CW], bf, tag="sA")
                    if unit in a_units:
                        nc.scalar.activation(sA[:], ps[:], A.Relu,
                                             bias=ncol, scale=1.0,
                                             accum_out=racc[:, unit:unit + 1])
                    else:
                        nc.vector.scalar_tensor_tensor(
                            sA[:], ps[:], mcol, zeros[:],
                            OP.subtract, OP.max,
                            accum_out=racc[:, unit:unit + 1])
                    nc.vector.tensor_scalar(gbufs[ch][:, j2 * CW:(j2 + 1) * CW],
                                            sA[:], 0.0, None, OP.is_gt)

                # same-class correction over this row-chunk's sorted window
                wps = psum.tile([128, CW], mybir.dt.float32, tag="ps")
                nc.tensor.matmul(wps[:, 0:W], lhsT,
                                 xtw_sb[:, ch * W:(ch + 1) * W],
                                 start=True, stop=True)
                uw = scratch.tile([128, W], bf, tag="uw")
                nc.scalar.activation(uw[:], wps[:, 0:W], A.Relu,
                                     bias=ncol, scale=1.0)
                eqs = eqm_sb[:, ch * W:(ch + 1) * W]
                jk1 = scratch.tile([128, W], bf, tag="jk1")
                nc.vector.scalar_tensor_tensor(
                    jk1[:], uw[:], 1.0, eqs, OP.mult, OP.mult,
                    accum_out=wacc[:, 2 * ch:2 * ch + 1])
                jk2 = scratch.tile([128, W], bf, tag="jk2")
                nc.vector.scalar_tensor_tensor(
                    jk2[:], uw[:], 0.0, eqs, OP.is_gt, OP.mult,
                    accum_out=wacc[:, 2 * ch + 1:2 * ch + 2])

                nc.vector.tensor_scalar(st_all[:, ch, :], io_sb[:],
                                        t_sb[:, ch:ch + 1], None, OP.is_equal)

            for q in range(N // 512):
                emit_cnt(NCH - 1, q)

            mg_sb = consts.tile([1, 512], f32)
            nc.scalar.copy(mg_sb[:], accm[:])

        # partial class-sum vectors V_k = S^T X over this core's rows
        v_sb = consts.tile([C, 128], f32)
        with tc.tile_pool(name="psB", bufs=1, space="PSUM") as vpsum:
            v_ps = vpsum.tile([C, 128], mybir.dt.float32)
            for ch in range(NCH):
                nc.tensor.matmul(v_ps[:], st_all[:, ch, :], xs_sb[:, ch, :],
                                 start=(ch == 0), stop=(ch == NCH - 1))
            nc.scalar.copy(v_sb[:], v_ps[:])

        dma.dma_start(out=out_v[:], in_=v_sb[:])
        dma.dma_start(out=out_racc[:], in_=racc[:])
        dma.dma_start(out=out_wacc[:], in_=wacc[:])
        dma.dma_start(out=out_mg[:], in_=mg_sb[:])

    nc.compile()
    return nc


def _prep(inputs, margin, targets):
    """Host-side sharding/layout prep. Returns per-core input maps + reduction data."""
    t = np.asarray(targets).astype(np.int64)
    x = np.asarray(inputs, dtype=np.float32)
    m = np.asarray(margin, dtype=np.float32)

    perm = np.argsort(t, kind="stable")
    xs, ms, ts = x[perm], m[perm], t[perm]
    x_bf = xs.astype(BF16)
    xt_bf = np.ascontiguousarray(x_bf.T)          # [128, N]

    cnt = np.bincount(ts, minlength=C).astype(np.float64)
    starts = np.concatenate([[0], np.cumsum(np.bincount(ts, minlength=C))]).astype(np.int64)

    nchunks = N // 128
    wstart = np.zeros(nchunks, np.int64)
    for g in range(nchunks):
        lo, hi = ts[g * 128], ts[g * 128 + 127]
        width = starts[hi + 1] - starts[lo]
        assert width <= W - 2, f"class window {width} too wide for chunk {g}"
        w0 = min(int(starts[lo]), N - W) & ~1
        wstart[g] = w0

    iotab = np.ascontiguousarray(
        np.broadcast_to(np.arange(C, dtype=np.float32), (128, C)))

    in_maps = []
    mrows = []
    for k in range(M):
        r0 = k * RPC
        g0 = r0 // 128
        mrow = np.ascontiguousarray(ms[r0:r0 + RPC].reshape(NCH, 128).T)
        trowf = np.ascontiguousarray(ts[r0:r0 + RPC].reshape(NCH, 128).T.astype(np.float32))
        xtw = np.concatenate(
            [xt_bf[:, wstart[g0 + ch]:wstart[g0 + ch] + W] for ch in range(NCH)], axis=1)
        eqm = np.concatenate(
            [(ts[r0 + ch * 128:r0 + (ch + 1) * 128, None]
              == ts[None, wstart[g0 + ch]:wstart[g0 + ch] + W]).astype(BF16)
             for ch in range(NCH)], axis=1)
        in_maps.append({
            "xt": xt_bf,
            "xtl": np.ascontiguousarray(xt_bf[:, r0:r0 + RPC]),
            "xtw": np.ascontiguousarray(xtw),
            "xsr": np.ascontiguousarray(x_bf[r0:r0 + RPC]),
            "mrow": mrow,
            "trow": trowf,
            "eqm": np.ascontiguousarray(eqm),
            "iotab": iotab,
        })
        mrows.append(mrow.astype(np.float64))
    return in_maps, mrows, cnt


def kernel(inputs, margin, targets):
    global _nc_cache, LAST_RESULTS
    in_maps, mrows, cnt = _prep(inputs, margin, targets)
    if _nc_cache is None:
        _nc_cache = _build_nc()
    res = run_bass_kernel_spmd(_nc_cache, in_maps, list(range(M)))
    LAST_RESULTS = res

    neg = 0.0
    V = np.zeros((C, 128), np.float64)
    for k in range(M):
        r = res.results[k]
        mr = mrows[k]                                  # [128, NCH] fp64
        neg += r["out_racc"].astype(np.float64).sum()  # sum relu(sim - m)
        neg += r["out_mg"].astype(np.float64).sum()    # sum m_i * [sim > m_i]
        wa = r["out_wacc"].astype(np.float64)          # [128, 2*NCH]
        neg -= wa[:, 0::2].sum()                       # same-class relu correction
        neg -= (mr * wa[:, 1::2]).sum()                # same-class m*count correction
        V += r["out_v"].astype(np.float64)

    pos = (cnt ** 2).sum() - (V ** 2).sum()
    loss = (pos + neg) / N
    return np.float32(loss)
